# revision 41
# baseline (speedup 1.0000x reference)
# 2D DCT-II [4096,4096] fp32 on 8 NeuronCores — v5 "two-level fold".
#
# v4 (bf16, SBUF-resident weights, split/overlapped AllToAll) left the PE
# 92% busy, so v5 cuts PE work itself: the even/odd DCT fold is applied
# RECURSIVELY to the even branch, splitting each pass into
#   odd   : U[2e+1]  = Wo  [2048x2048] @ (x[r] - x[~r])           (16 k-tiles)
#   even-a: U[4e2]   = We2a[1024x1024] @ (xp[r2] + xp[~r2])        (8 k-tiles)
#   even-b: U[4e2+2] = We2b[1024x1024] @ (xp[r2] - xp[~r2])        (8 k-tiles)
# for 6M MACs per output column instead of 8M (384 matmuls/pass vs 512) and
# 12.6MB of resident weights instead of 16.8MB. The partition-reversed
# mirrors (x[~r]) come from tiny J-matmuls against the anti-identity.
#
# Orientation: data tiles are lhsT (stationary), cos-weights are rhs
# (moving); pass-1 emits U^T tiles [c-part, row-free]; the AllToAll is split
# into even/odd halves (z1a/z1b), each hidden under the next compute block;
# pass-2 consumes the A2A output directly, with all fold prep interleaved
# into the preceding matmul block so the PE never idles at boundaries.
# Core c owns true rows 512c..512c+511: z1a[c] carries them as
# j<128 -> row 512c+4j (branch a), j>=128 -> 512c+4(j-128)+2 (branch b);
# z1b[c]: j -> 512c+2j+1. Queue placement: input/weight streaming on SP in
# first-use order, staging loads + tr2 + drain y stores on ACT, z1 stores +
# collectives + overlapped y stores on Pool/SWDGE (separate semaphore pool),
# folds/evac on DVE; a 12-deep ev pool absorbs z1-store latency behind the
# weight stream.
import numpy as np
import ml_dtypes
from einops import rearrange
import concourse.bacc as bacc
import concourse.tile as tile
import concourse.mybir as mybir
from concourse import bass_utils

M = N = 4096
NC = 8
CB = 512          # columns per core (pass 1) / rows per core (pass 2)
KH = M // 2       # 2048 level-1 folded contraction length
KT = KH // 128    # 16 K-tiles (odd branch)
KT2 = KT // 2     # 8 K-tiles (level-2 even branches)
NCH = KH // 512   # 4 odd-branch N-chunks of 512

_BUILT = {}


def build_nc(repeat=1, local_sim=False):
    dt = mybir.dt
    bf = dt.bfloat16
    nc = bacc.Bacc("TRN2", target_bir_lowering=False, debug=False, num_devices=NC)

    # packed pass-1 input, mirror-pair bundles: chunk c4 carries, for its
    # two pairs q = 2*c4 + j2, the four planes (Xf[q], Xr[q], Xf[15-q],
    # Xr[15-q]) at e = 4*j2 + (0..3) — so each chunk feeds the complete
    # level-1 AND level-2 fold chain for its k-pairs with no cross-chunk wait.
    xfr = nc.dram_tensor("xfr", [128, 4, 8, CB], bf, kind="ExternalInput")
    wo = nc.dram_tensor("wo", [128, NCH, KT, 512], bf, kind="ExternalInput")
    we2a = nc.dram_tensor("we2a", [128, KT2, 1024], bf, kind="ExternalInput")
    we2b = nc.dram_tensor("we2b", [128, KT2, 1024], bf, kind="ExternalInput")
    jrev = nc.dram_tensor("jrev", [128, 128], bf, kind="ExternalInput")
    y = nc.dram_tensor("y", [CB, N], bf, kind="ExternalOutput")

    with tile.TileContext(nc) as tc:
        with (
            tc.tile_pool(name="dram", bufs=1, space="DRAM") as dram,
            tc.tile_pool(name="wpool", bufs=1) as wpool,
            tc.tile_pool(name="foldp", bufs=1) as foldp,
            tc.tile_pool(name="xst", bufs=2) as xst,
            tc.tile_pool(name="zst", bufs=4) as zst,
            tc.tile_pool(name="evp", bufs=12) as evp,
            tc.tile_pool(name="jp", bufs=1) as jp,
            tc.tile_pool(name="ytp", bufs=1) as ytp,
            tc.tile_pool(name="psp", bufs=4, space="PSUM") as psp,
            tc.tile_pool(name="psj", bufs=3, space="PSUM") as psj,
        ):
            z1a = dram.tile([NC, CB, 256], bf)
            z1b = dram.tile([NC, CB, 256], bf)
            z2a = dram.tile([NC, CB, 256], bf)
            z2b = dram.tile([NC, CB, 256], bf)

            for _rep in range(repeat):
                jt = jp.tile([128, 128], bf, tag="jt")
                nc.scalar.dma_start(out=jt[:], in_=jrev[:])
                # ---- streaming on SP in strict first-use order
                w2a = wpool.tile([128, KT2, 1024], bf, tag="w2a", name="w2a")
                w2b = wpool.tile([128, KT2, 1024], bf, tag="w2b", name="w2b")
                wos = [wpool.tile([128, KT, 512], bf, tag=f"wo{i}", name="wos")
                       for i in range(NCH)]
                xcs = []
                for c4 in range(4):
                    xc = xst.tile([128, 8, CB], bf, tag="xc", name="xc")
                    nc.sync.dma_start(out=xc[:], in_=xfr[:, c4])
                    xcs.append(xc)
                # even-a's first groups need only w2a's first e2-half;
                # splitting lets them start ~3us earlier
                nc.sync.dma_start(out=w2a[:, :, 0:512], in_=we2a[:, :, 0:512])
                nc.sync.dma_start(out=w2a[:, :, 512:1024],
                                  in_=we2a[:, :, 512:1024])
                nc.sync.dma_start(out=w2b[:], in_=we2b[:])
                nc.sync.dma_start(out=wos[0][:], in_=wo[:, 0])
                nc.sync.dma_start(out=wos[1][:], in_=wo[:, 1])
                # ---- folds, fully pipelined per mirror-pair chunk:
                # level-1: xp = X[r]+X[4095-r], xm = X[r]-X[4095-r];
                # level-2 on the even branch (r2 mirror via J-matmul):
                # xp2[:, k2, 0:512] = xp[r2]+xp[2047-r2], [512:1024] = minus.
                xp = foldp.tile([128, KT, CB], bf, tag="fA", name="xp")
                xm = foldp.tile([128, KT, CB], bf, tag="fB", name="xm")
                xp2 = foldp.tile([128, KT2, 1024], bf, tag="f2", name="xp2")
                for c4 in range(4):
                    xc = xcs[c4]
                    for j2 in range(2):
                        q = 2 * c4 + j2
                        b = 4 * j2
                        nc.vector.tensor_add(xp[:, q], xc[:, b], xc[:, b + 1])
                        nc.vector.tensor_add(xp[:, KT - 1 - q],
                                             xc[:, b + 2], xc[:, b + 3])
                        nc.vector.tensor_sub(xm[:, q], xc[:, b], xc[:, b + 1])
                        nc.vector.tensor_sub(xm[:, KT - 1 - q],
                                             xc[:, b + 2], xc[:, b + 3])
                        prr = psj.tile([128, 512], dt.float32, tag="pj",
                                       name="prr")
                        nc.tensor.matmul(prr[:], jt[:], xp[:, KT - 1 - q],
                                         start=True, stop=True)
                        xq = zst.tile([128, 512], bf, tag="xq", name="xq",
                                      bufs=3)
                        nc.scalar.copy(xq[:], prr[:])
                        nc.vector.tensor_add(xp2[:, q, 0:512], xp[:, q], xq[:])
                        nc.vector.tensor_sub(xp2[:, q, 512:1024],
                                             xp[:, q], xq[:])
                # wo2/wo3 aren't needed until deep into the odd block; riding
                # ACT behind the J2 xq copies delays their dispatch ~20us so
                # they don't starve the front (x, w2a/b, wo0/1, z1a stores)
                nc.scalar.dma_start(out=wos[2][:], in_=wo[:, 2])
                nc.scalar.dma_start(out=wos[3][:], in_=wo[:, 3])

                # ---- pass-2 prep emitters (interleaved into the preceding
                # matmul block). Level-1: paired staging loads + J-reversal +
                # fold into fz; level-2: J-reversal of fz's zp half into fz2.
                def emit_load(z2x, stage, kt):
                    zr = z2x[:].rearrange("s (ch p) j -> p (s ch) j", p=128)
                    tf2 = zst.tile([128, 2, 256], bf, tag="tf2", name="tf2",
                                   bufs=8)
                    ta2 = zst.tile([128, 2, 256], bf, tag="ta2", name="ta2",
                                   bufs=8)
                    nc.scalar.dma_start(out=tf2[:], in_=zr[:, kt:kt + 2])
                    nc.scalar.dma_start(out=ta2[:], in_=zr[:, 30 - kt:32 - kt])
                    stage[kt] = (tf2, ta2)

                def emit_jfold(fz, stage, kt):
                    tf2, ta2 = stage[kt]
                    prr = psj.tile([128, 512], dt.float32, tag="pj", name="prr")
                    nc.tensor.matmul(prr[:], jt[:],
                                     ta2[:].rearrange("p t j -> p (t j)"),
                                     start=True, stop=True)
                    tr2 = zst.tile([128, 2, 256], bf, tag="tr2", name="tr2")
                    nc.scalar.copy(tr2[:].rearrange("p t j -> p (t j)"), prr[:])
                    for t in range(2):
                        nc.vector.tensor_add(fz[:, kt + t, 0:256],
                                             tf2[:, t], tr2[:, 1 - t])
                        nc.vector.tensor_sub(fz[:, kt + t, 256:512],
                                             tf2[:, t], tr2[:, 1 - t])

                def emit_jfold2(fz, fz2, k2):
                    prr = psj.tile([128, 256], dt.float32, tag="pj", name="prr")
                    nc.tensor.matmul(prr[:], jt[:], fz[:, KT - 1 - k2, 0:256],
                                     start=True, stop=True)
                    qz = zst.tile([128, 256], bf, tag="qz", name="qz", bufs=4)
                    nc.scalar.copy(qz[:], prr[:])
                    nc.vector.tensor_add(fz2[:, k2, 0:256],
                                         fz[:, k2, 0:256], qz[:])
                    nc.vector.tensor_sub(fz2[:, k2, 256:512],
                                         fz[:, k2, 0:256], qz[:])

                def hook(z2x, fz, fz2, stage, jg):
                    def run(g):
                        if 4 <= g < 8:
                            emit_load(z2x, stage, 4 * (g - 4))
                            emit_load(z2x, stage, 4 * (g - 4) + 2)
                        if jg <= g < jg + 4:
                            emit_jfold(fz, stage, 4 * (g - jg))
                            emit_jfold(fz, stage, 4 * (g - jg) + 2)
                        if jg + 4 <= g < jg + 6:
                            for k2 in range(4 * (g - jg - 4), 4 * (g - jg - 3)):
                                emit_jfold2(fz, fz2, k2)
                    return run

                # ================= pass 1, even branches (-> z1a) =========
                # branch a (+fold, We2a) then b (-fold, We2b); psum[c, e2]
                # splits 4 ways: dest core 4*nch2a+piece, j = br*128 + e2%128
                for br, wt2 in ((0, w2a), (1, w2b)):
                    for nch2a in range(2):
                        for cm in range(CB // 128):
                            psum = psp.tile([128, 512], dt.float32, tag="ps",
                                            name="ps1e")
                            for k2 in range(KT2):
                                nc.tensor.matmul(
                                    psum[:],
                                    xp2[:, k2, br * 512 + cm * 128:
                                        br * 512 + (cm + 1) * 128],
                                    wt2[:, k2, nch2a * 512:(nch2a + 1) * 512],
                                    start=(k2 == 0), stop=(k2 == KT2 - 1))
                            ev = evp.tile([128, 512], bf, tag="ev", name="ev")
                            nc.vector.tensor_copy(ev[:], psum[:])
                            for piece in range(4):
                                eng = nc.sync if piece % 2 == 0 else nc.scalar
                                eng.dma_start(
                                    out=z1a[4 * nch2a + piece,
                                            cm * 128:(cm + 1) * 128,
                                            br * 128:(br + 1) * 128],
                                    in_=ev[:, piece * 128:(piece + 1) * 128])
                if local_sim:
                    nc.gpsimd.dma_start(out=z2a[:], in_=z1a[:])
                else:
                    nc.gpsimd.collective_compute(
                        "AllToAll", mybir.AluOpType.bypass,
                        replica_groups=[list(range(NC))],
                        ins=[z1a[:].opt()], outs=[z2a[:].opt()])

                # ================= pass 1, odd branch (-> z1b) ============
                fza = foldp.tile([128, KT, CB], bf, tag="fA", name="fza")
                fz2a = foldp.tile([128, KT2, 1024], bf, tag="f2", name="fz2a")
                stage0, stage1 = {}, {}
                prep = hook(z2a, fza, fz2a, stage0, 9)
                g = 0
                for nch in range(NCH):
                    for cm in range(CB // 128):
                        psum = psp.tile([128, 512], dt.float32, tag="ps",
                                        name="ps1o")
                        for k in range(KT):
                            nc.tensor.matmul(psum[:],
                                             xm[:, k, cm * 128:(cm + 1) * 128],
                                             wos[nch][:, k],
                                             start=(k == 0), stop=(k == KT - 1))
                        ev = evp.tile([128, 512], bf, tag="ev", name="ev")
                        nc.vector.tensor_copy(ev[:], psum[:])
                        for piece in range(2):
                            nc.gpsimd.dma_start(
                                out=z1b[2 * nch + piece,
                                        cm * 128:(cm + 1) * 128, :],
                                in_=ev[:, piece * 256:(piece + 1) * 256])
                        prep(g)
                        g += 1
                if local_sim:
                    nc.gpsimd.dma_start(out=z2b[:], in_=z1b[:])
                else:
                    nc.gpsimd.collective_compute(
                        "AllToAll", mybir.AluOpType.bypass,
                        replica_groups=[list(range(NC))],
                        ins=[z1b[:].opt()], outs=[z2b[:].opt()])

                # ================= pass 2 =================
                # per fhalf: 256 owned rows (batch dim j); contraction over
                # original columns c_g, folded once (odd) or twice (even).
                yv4 = y[:].rearrange("(a b) n -> a b n", b=4)  # [128, 4, 4096]
                yv2 = y[:].rearrange("(a b) n -> a b n", b=2)  # [256, 2, 4096]
                fzb = fz2b = None
                for fhalf in range(2):
                    if fhalf == 0:
                        fz, fz2 = fza, fz2a
                        fzb = foldp.tile([128, KT, CB], bf, tag="fB", name="fzb")
                        fz2b = foldp.tile([128, KT2, 1024], bf, tag="f2",
                                          name="fz2b")
                        prep = hook(z2b, fzb, fz2b, stage1, 10)
                    else:
                        fz, fz2 = fzb, fz2b
                        prep = None
                    g = 0
                    for rm in range(2):
                        for h in range(2):
                            # half-row buffer: true cols [2048h, 2048h+2048)
                            yt = ytp.tile([128, N // 2], bf, tag=f"yt{rm}",
                                          name="yt")
                            yt2 = yt[:].rearrange("p (a b) -> p a b", b=2)
                            yt4 = yt[:].rearrange("p (a b) -> p a b", b=4)
                            specs = [
                                ("o", 2 * h), ("o", 2 * h + 1),
                                ("a", h), ("b", h),
                            ]
                            for kind, idx in specs:
                                psum = psp.tile([128, 512], dt.float32,
                                                tag="ps", name="ps2")
                                if kind == "o":
                                    for k in range(KT):
                                        nc.tensor.matmul(
                                            psum[:],
                                            fz[:, k, 256 + rm * 128:
                                               256 + (rm + 1) * 128],
                                            wos[idx][:, k],
                                            start=(k == 0), stop=(k == KT - 1))
                                    # true col = 2e+1, e = idx*512 + q
                                    d = idx - 2 * h
                                    nc.vector.tensor_copy(
                                        yt2[:, d * 512:(d + 1) * 512, 1],
                                        psum[:])
                                else:
                                    woff = 0 if kind == "a" else 256
                                    wt2 = w2a if kind == "a" else w2b
                                    for k2 in range(KT2):
                                        nc.tensor.matmul(
                                            psum[:],
                                            fz2[:, k2, woff + rm * 128:
                                                woff + (rm + 1) * 128],
                                            wt2[:, k2,
                                                idx * 512:(idx + 1) * 512],
                                            start=(k2 == 0),
                                            stop=(k2 == KT2 - 1))
                                    # true col = 4e2 (+2 for branch b)
                                    nc.vector.tensor_copy(
                                        yt4[:, :, 0 if kind == "a" else 2],
                                        psum[:])
                                if prep is not None:
                                    prep(g)
                                g += 1
                            if fhalf == 0:
                                # rm0 -> rows 4j, rm1 -> rows 4j+2 (SWDGE:
                                # off the HWDGE rotation, can't throttle the
                                # f1 staging loads)
                                nc.gpsimd.dma_start(
                                    out=yv4[:, 2 * rm,
                                            h * 2048:(h + 1) * 2048],
                                    in_=yt[:])
                            else:
                                # rows 2(rm*128+p)+1; fast HWDGE for drain
                                nc.scalar.dma_start(
                                    out=yv2[rm * 128:(rm + 1) * 128, 1,
                                            h * 2048:(h + 1) * 2048],
                                    in_=yt[:])

    nc.compile()
    return nc


def _weights():
    r = np.arange(KH, dtype=np.float64)
    e = np.arange(KH, dtype=np.float64)
    Wo = np.cos(np.pi * (2.0 * r[:, None] + 1.0) * (2.0 * e[None, :] + 1.0)
                / (2.0 * M))                        # [r, e] odd branch
    r2 = np.arange(1024, dtype=np.float64)
    e2 = np.arange(1024, dtype=np.float64)
    We2a = np.cos(np.pi * (2.0 * r2[:, None] + 1.0) * e2[None, :] / 2048.0)
    We2b = np.cos(np.pi * (2.0 * r2[:, None] + 1.0) * (2.0 * e2[None, :] + 1.0)
                  / 4096.0)
    return Wo, We2a, We2b


def tile3(a):
    return np.ascontiguousarray(rearrange(a, "(m p) n -> p m n", p=128))


def _host_inputs():
    bf = ml_dtypes.bfloat16
    Wo, We2a, We2b = _weights()
    wo4 = tile3(Wo).reshape(128, KT, NCH, 512).transpose(0, 2, 1, 3)
    return {
        "wo": np.ascontiguousarray(wo4).astype(bf),
        "we2a": tile3(We2a).astype(bf),
        "we2b": tile3(We2b).astype(bf),
        "jrev": np.ascontiguousarray(np.eye(128)[::-1]).astype(bf),
    }


def kernel(x, expkM=None, expkN=None, trace=False):
    bf = ml_dtypes.bfloat16
    x = np.asarray(x, dtype=np.float32).astype(bf)
    if "nc" not in _BUILT:
        _BUILT["nc"] = build_nc()
        _BUILT.update(_host_inputs())
    nc = _BUILT["nc"]
    xrev = x[::-1, :]
    in_maps = []
    for c in range(NC):
        sl = slice(c * CB, (c + 1) * CB)
        xf_t = tile3(x[:KH, sl])      # [128, KT, CB]
        xr_t = tile3(xrev[:KH, sl])
        xfr = np.empty((128, 4, 8, CB), dtype=xf_t.dtype)
        for c4 in range(4):
            for j2 in range(2):
                q = 2 * c4 + j2
                xfr[:, c4, 4 * j2 + 0] = xf_t[:, q]
                xfr[:, c4, 4 * j2 + 1] = xr_t[:, q]
                xfr[:, c4, 4 * j2 + 2] = xf_t[:, KT - 1 - q]
                xfr[:, c4, 4 * j2 + 3] = xr_t[:, KT - 1 - q]
        in_maps.append({
            "xfr": np.ascontiguousarray(xfr),
            "wo": _BUILT["wo"],
            "we2a": _BUILT["we2a"],
            "we2b": _BUILT["we2b"],
            "jrev": _BUILT["jrev"],
        })
    res = bass_utils.run_bass_kernel_spmd(nc, in_maps, core_ids=list(range(NC)),
                                          trace=trace)
    _BUILT["last_res"] = res
    out = np.concatenate([res.results[c]["y"] for c in range(NC)], axis=0)
    return out.astype(np.float32)
